# revision 32
# baseline (speedup 1.0000x reference)
"""Causal GQA self-attention block (B=4, T=2048, C=1024, H=16, G=4) on 8
Trainium2 NeuronCores.

Sharding: core c = d*4+g  (d in {0,1} batch-DP, g in {0..3} kv-group TP).
Each core handles batches [2d, 2d+1], heads {g, g+4, g+8, g+12}, kv group g,
and produces a partial projection output; the host sums the 4 TP partials
per batch pair and adds the bias.

Per-core kernel (fp16 operands, fp32 PSUM accumulation):
  - fused QKV projection from pre-transposed x (host supplies x^T in fp16),
    x^T staged as 8 per-chunk tiles so the first projection chain starts as
    soon as chunk 0 lands (kills the startup DMA serialization)
  - scores computed transposed (S^T[tk,tq] = K Q^T) in 128x512 tiles,
    head-pair packed into the PE array via tile_position (contraction=64)
  - causal: block skip + column trim + multiplicative 0/1 band mask on the
    diagonal blocks (applied post-exp on DVE)
  - unnormalized softmax: exp on ACT (scale folded), denominator obtained
    by appending a ones-column to V in the P@V matmul (M=65)
  - V^T -> V transposes via the DMA XBAR (off the PE/DVE)
  - normalize via DVE reciprocal (both heads in one [2,NT] call) + gpsimd
    partition-broadcast + DVE mult
  - output projection on-device, fp16 partials DMA'd out; host sums in fp32
  - both batches emitted as one flat pipeline so batch 1's attention starts
    while batch 0's trailing projection chunks still run
"""

import os
import sys

sys.path.insert(0, "/opt/trn_rl_repo")

import numpy as np
from contextlib import ExitStack

import concourse.bass as bass
import concourse.mybir as mybir
import concourse.tile as tile
from concourse import bacc
from concourse.bass_utils import run_bass_kernel_spmd

# problem shape (hardcoded per contract)
B, T, C = 4, 2048, 1024
H, G = 16, 4
D = C // H  # 64

# per-core
B_LOC = 2        # batches per core
NPAIR = 2        # head pairs per core (4 heads)
P = 128
CC = C // P      # 8 contraction chunks for projections
NT = 512         # tq tile width
TQT = T // NT    # 4 tq tiles
TKC = T // P     # 16 tk chunks

F32 = mybir.dt.float32
F16 = mybir.dt.float16
ADT = F16
Exp = mybir.ActivationFunctionType.Exp
ADD = mybir.AluOpType.add
MULT = mybir.AluOpType.mult


def _build_program():
    nc = bacc.Bacc(None, target_bir_lowering=False)

    xT = nc.dram_tensor("xT", [B_LOC, C, T], ADT, kind="ExternalInput")
    # pre-swizzled on host so each SBUF partition's data is contiguous in
    # DRAM (dense 6KB/4KB descriptors instead of a 768B trickle):
    # wqkv[p, cc, :] = WqkvT[cc*128+p, :] with columns q0|q1|k|v
    wqkv = nc.dram_tensor("wqkv", [P, CC, 384], ADT, kind="ExternalInput")
    wproj = nc.dram_tensor("wproj", [P, 2, C], ADT, kind="ExternalInput")
    # multiplicative triangular band mask, duplicated for the 2 packed heads
    maskb = nc.dram_tensor("maskb", [P, 2, P], ADT, kind="ExternalInput")
    ident2 = nc.dram_tensor("ident2", [P, 64], ADT, kind="ExternalInput")
    outp = nc.dram_tensor("outp", [B_LOC, T, C], F16, kind="ExternalOutput")

    with tile.TileContext(nc) as tc:
        with ExitStack() as ctx:
            const = ctx.enter_context(tc.tile_pool(name="const", bufs=1))
            xp = ctx.enter_context(tc.tile_pool(name="xp", bufs=2))
            sb2 = ctx.enter_context(tc.tile_pool(name="sb2", bufs=2))
            small = ctx.enter_context(tc.tile_pool(name="small", bufs=4))
            ppool = ctx.enter_context(tc.tile_pool(name="ppool", bufs=4))
            pvsp = ctx.enter_context(tc.tile_pool(name="pvsp", bufs=2))
            stg = ctx.enter_context(tc.tile_pool(name="stg", bufs=3))
            ps_st = ctx.enter_context(tc.tile_pool(name="ps_st", bufs=2, space="PSUM"))
            ps_pv = ctx.enter_context(tc.tile_pool(name="ps_pv", bufs=2, space="PSUM"))
            ps_mm = ctx.enter_context(tc.tile_pool(name="ps_mm", bufs=2, space="PSUM"))

            # ---- constants (single strided DMAs to cut issue latency) ----
            wqkv_t = const.tile([P, CC, 384], ADT, tag="wqkv")
            wproj_t = const.tile([P, 2, C], ADT, tag="wproj")
            mask_t = const.tile([P, 2, P], ADT, tag="maskb")
            id2_t = const.tile([P, 64], ADT, tag="ident2")

            def emit_consts_late():
                nc.scalar.dma_start(wproj_t[:], wproj[:])
                nc.scalar.dma_start(id2_t[:], ident2[:])

            def emit_setup(b, first=False):
                if first:
                    # the first projection chains need wqkv before anything
                    nc.scalar.dma_start(wqkv_t[:], wqkv[:])
                    nc.gpsimd.dma_start(mask_t[:], maskb[:])
                xts = []
                for cc in range(CC):
                    xt_c = xp.tile([P, T], ADT, tag=f"xt{cc}", name=f"xt{b}_{cc}")
                    eng = (nc.sync, nc.gpsimd, nc.scalar)[cc % 3]
                    eng.dma_start(xt_c[:], xT[b, cc * P : (cc + 1) * P, :])
                    xts.append(xt_c)
                if first:
                    emit_consts_late()
                # q_sb[:, p, t]: pair p -> heads (2p, 2p+1) at rows 0:64 / 64:128
                q_sb = sb2.tile([P, NPAIR, T], ADT, tag="q", name=f"q{b}")
                # kv_sb rows 0:64 = K^T (kv-group), rows 64:128 = V^T
                kv_sb = sb2.tile([P, TQT, NT], ADT, tag="kv", name=f"kv{b}")
                k_hi = sb2.tile([P, TQT, NT], ADT, tag="khi", name=f"khi{b}")
                v_a = sb2.tile([P, TKC, 65], ADT, tag="va", name=f"va{b}")
                nc.vector.memset(v_a[:, :, 64], 1.0)
                o_t = sb2.tile([P, NPAIR, T], ADT, tag="ot", name=f"ot{b}")
                return xts, q_sb, kv_sb, k_hi, v_a, o_t

            def emit_qkv_part(b, st8, n, part):
                # ---- QKV projection tile n, sub-part (0: kv proj + V
                # transpose + k dup, 1: q pair0 proj, 2: q pair1 proj) ----
                xts, q_sb, kv_sb, k_hi, v_a, o_t = st8
                m = {0: 2, 1: 0, 2: 1}[part]
                pm = ps_mm.tile([P, NT], F32, tag="mm")
                for cc in range(CC):
                    nc.tensor.matmul(
                        pm[:],
                        wqkv_t[:, cc, m * P : (m + 1) * P],
                        xts[cc][:, n * NT : (n + 1) * NT],
                        start=(cc == 0),
                        stop=(cc == CC - 1),
                    )
                if m < 2:
                    nc.vector.tensor_copy(q_sb[:, m, n * NT : (n + 1) * NT], pm[:])
                    # V transposes deferred here so they don't sit ahead of
                    # the q chains (and the first scores) in the PE queue
                    for i in range(4 * n + 2 * m, 4 * n + 2 * m + 2):
                        pt = ps_mm.tile([P, 64], ADT, tag="mm")
                        nc.tensor.transpose(
                            pt[:],
                            kv_sb[64:128, i // 4, (i % 4) * P : (i % 4 + 1) * P],
                            id2_t[64:128, :],
                        )
                        nc.vector.tensor_copy(v_a[:, i, 0:64], pt[:])
                    return
                nc.vector.tensor_copy(kv_sb[:, n, :], pm[:])
                nc.sync.dma_start(k_hi[64:128, n, :], kv_sb[0:64, n, :])

            def emit_attn_jp(b, st8, j, p_, fills=()):
                xts, q_sb, kv_sb, k_hi, v_a, o_t = st8
                fills = list(fills)
                pv = [
                    ps_pv.tile([P, NT], F32, tag="pv", name=f"pv{b}{j}{p_}{e}")
                    for e in range(2)
                ]
                last = 4 * j + 3
                for i in range(4 * j + 4):
                    # sprinkle fill chains between blocks so the ACT stream
                    # always has the next scores ready
                    if i >= 2 and i % 2 == 0 and fills:
                        fills.pop(0)()
                    diag = i >= 4 * j
                    r = i - 4 * j
                    lo = r * P if diag else 0
                    st = ps_st.tile([P, 2, NT], F32, tag="st")
                    for e in range(2):
                        ksrc = kv_sb if e == 0 else k_hi
                        nc.tensor.matmul(
                            st[:, e, lo:NT],
                            ksrc[
                                64 * e : 64 * e + 64,
                                i // 4,
                                (i % 4) * P : (i % 4 + 1) * P,
                            ],
                            q_sb[
                                64 * e : 64 * e + 64,
                                p_,
                                j * NT + lo : (j + 1) * NT,
                            ],
                            start=True,
                            stop=True,
                            tile_position=(64 * e, 0),
                        )
                    pexp = ppool.tile([P, 2, NT], ADT, tag="pexp")
                    nc.scalar.activation(
                        pexp[:, :, lo:NT],
                        st[:, :, lo:NT],
                        Exp,
                        scale=0.125,
                    )
                    if diag:
                        nc.vector.tensor_tensor(
                            pexp[:, :, lo : lo + P],
                            pexp[:, :, lo : lo + P],
                            mask_t[:],
                            MULT,
                        )
                    for e in range(2):
                        nc.tensor.matmul(
                            pv[e][0:65, lo:NT],
                            v_a[:, i, :],
                            pexp[:, e, lo:NT],
                            start=(i == 0),
                            stop=(i == last),
                        )
                # normalize: o = pv[0:64] / pv[64].  The PSUM->SBUF copies run
                # on ACT so the pv banks release without waiting on the DVE
                # queue; both heads' reciprocals batch into one [2, NT] call.
                pvs = pvsp.tile([65, 2, NT], F32, tag="pvs", name=f"pvs{b}{j}{p_}")
                for e in range(2):
                    nc.vector.tensor_copy(pvs[:, e, :], pv[e][0:65, :])
                # reciprocal_approx_fast and partition_broadcast require
                # absolute partition 0 on HW: shift denominator rows down
                l0 = small.tile([2, NT], F32, tag="l0")
                nc.sync.dma_start(l0[:], pvs[64:65, :, :])
                rec = small.tile([2, NT], F32, tag="rec")
                nc.vector.reciprocal_approx_fast(rec[:], l0[:])
                r1 = small.tile([1, NT], F32, tag="r1")
                nc.sync.dma_start(r1[:], rec[1:2, :])
                bca0 = small.tile([64, NT], F32, tag="bca0")
                nc.gpsimd.partition_broadcast(bca0[:], rec[0:1, :])
                bca1 = small.tile([64, NT], F32, tag="bca1")
                nc.gpsimd.partition_broadcast(bca1[:], r1[:])
                nc.vector.tensor_tensor(
                    o_t[0:64, p_, j * NT : (j + 1) * NT],
                    pvs[0:64, 0, :],
                    bca0[:],
                    MULT,
                )
                otmp = small.tile([64, NT], ADT, tag="otmp")
                nc.vector.tensor_tensor(otmp[:], pvs[0:64, 1, :], bca1[:], MULT)
                nc.sync.dma_start(o_t[64:128, p_, j * NT : (j + 1) * NT], otmp[:])
                for f in fills:
                    f()

            def emit_proj_t(b, st8, t_):
                # ---- output projection for one tq chunk (fp16 partial) ----
                o_t = st8[5]
                stage = stg.tile([P, C], ADT, tag="stage")
                for n2 in range(2):
                    pm = ps_mm.tile([P, NT], F32, tag="mm")
                    for cc2 in range(2):
                        nc.tensor.matmul(
                            pm[:],
                            o_t[:, cc2, t_ * P : (t_ + 1) * P],
                            wproj_t[:, cc2, n2 * NT : (n2 + 1) * NT],
                            start=(cc2 == 0),
                            stop=(cc2 == 1),
                        )
                    # fp16 staging: ACT's copy precision is moot, and this
                    # keeps the DVE free for the normalize chains
                    nc.scalar.copy(stage[:, n2 * NT : (n2 + 1) * NT], pm[:])
                nc.gpsimd.dma_start(outp[b, t_ * P : (t_ + 1) * P, :], stage[:])

            # ---- flat two-batch pipeline ----
            QK = lambda b, n, p: ("qkv", b, n, p)
            PR = lambda b, t: ("proj", b, t)
            SU = lambda b: ("setup", b)
            plan = [
                (0, 0, 0, [QK(0, 1, 0)]),
                (0, 0, 1, [QK(0, 1, 1), QK(0, 1, 2)]),
                (0, 1, 0, [QK(0, 2, 0), QK(0, 2, 1)]),
                (0, 1, 1, [QK(0, 2, 2), QK(0, 3, 0), PR(0, 0), PR(0, 1)]),
                (0, 2, 0, [QK(0, 3, 1), QK(0, 3, 2), PR(0, 2), PR(0, 3)]),
                (0, 2, 1, [SU(1), QK(1, 0, 0), PR(0, 4)]),
                (0, 3, 0, [QK(1, 0, 1), QK(1, 0, 2), PR(0, 5)]),
                (0, 3, 1, [QK(1, 1, 0), PR(0, 6), PR(0, 7)]),
                (1, 0, 0, [QK(1, 1, 1), QK(1, 1, 2), PR(0, 8)]),
                (1, 0, 1, [QK(1, 2, 0), QK(1, 2, 1), PR(0, 9)]),
                (1, 1, 0, [QK(1, 2, 2), QK(1, 3, 0), PR(0, 10), PR(0, 11)]),
                (1, 1, 1, [QK(1, 3, 1), QK(1, 3, 2), PR(0, 12), PR(0, 13)]),
                (1, 2, 0, [PR(0, 14), PR(0, 15), PR(1, 0), PR(1, 1)]),
                (1, 2, 1, [PR(1, 2), PR(1, 3), PR(1, 4), PR(1, 5)]),
                (1, 3, 0, [PR(1, 6), PR(1, 7), PR(1, 8), PR(1, 9)]),
                (1, 3, 1, [PR(1, 10), PR(1, 11)]),
            ]
            st = {0: emit_setup(0, first=True)}
            for p in range(3):
                emit_qkv_part(0, st[0], 0, p)
            def mk_fill(f):
                if f[0] == "qkv":
                    return lambda: emit_qkv_part(f[1], st[f[1]], f[2], f[3])
                if f[0] == "proj":
                    return lambda: emit_proj_t(f[1], st[f[1]], f[2])
                def do_setup():
                    st[f[1]] = emit_setup(f[1])
                return do_setup

            for b, j, p_, fills in plan:
                emit_attn_jp(b, st[b], j, p_, [mk_fill(f) for f in fills])
            for t_ in range(12, 16):
                emit_proj_t(1, st[1], t_)

    nc.compile()
    return nc


_NC = None


def _get_program():
    global _NC
    if _NC is None:
        _NC = _build_program()
    return _NC


def _host_inputs(x, Wq, Wkv, Wproj):
    """Shard + lay out inputs for the 8 cores."""
    adt_np = np.float16
    tri = np.where(
        np.arange(P)[:, None] <= np.arange(P)[None, :], 1.0, 0.0
    ).astype(np.float32)
    maskb = np.stack([tri, tri], axis=1).astype(adt_np)  # [128, 2, 128]
    ident2 = np.concatenate([np.eye(64, dtype=np.float32)] * 2, axis=0).astype(
        adt_np
    )  # [128, 64]

    in_maps = []
    for d in range(2):
        xT = x[2 * d : 2 * d + 2].transpose(0, 2, 1).astype(adt_np)
        for g in range(G):
            heads = [g, g + 4, g + 8, g + 12]
            wq_cols = np.concatenate(
                [Wq[h * D : (h + 1) * D, :] for h in heads], axis=0
            ).T  # [1024, 256]
            wk = Wkv[g * D : (g + 1) * D, :].T  # [1024, 64]
            wv = Wkv[G * D + g * D : G * D + (g + 1) * D, :].T
            wqkv = np.concatenate([wq_cols, wk, wv], axis=1)  # [1024, 384]
            # swizzle: partition-major so each partition's rows are dense
            wqkv_s = np.ascontiguousarray(
                wqkv.reshape(CC, P, 384).transpose(1, 0, 2)
            ).astype(adt_np)  # [128, CC, 384]
            ch = np.concatenate(
                [np.arange(h * D, (h + 1) * D) for h in heads]
            )
            wproj_full = np.ascontiguousarray(Wproj[:, ch].T)  # [256, 1024]
            wproj_s = np.ascontiguousarray(
                wproj_full.reshape(2, P, C).transpose(1, 0, 2)
            ).astype(adt_np)  # [128, 2, 1024]
            in_maps.append(
                {
                    "xT": xT,
                    "wqkv": wqkv_s,
                    "wproj": wproj_s,
                    "maskb": maskb,
                    "ident2": ident2,
                }
            )
    return in_maps


def kernel(x, Wq, Wkv, Wproj, b_proj):
    x = np.asarray(x, dtype=np.float32)
    Wq = np.asarray(Wq, dtype=np.float32)
    Wkv = np.asarray(Wkv, dtype=np.float32)
    Wproj = np.asarray(Wproj, dtype=np.float32)
    b_proj = np.asarray(b_proj, dtype=np.float32)

    nc = _get_program()
    in_maps = _host_inputs(x, Wq, Wkv, Wproj)
    trace = bool(int(os.environ.get("BASS_KERNEL_TRACE", "0")))
    res = run_bass_kernel_spmd(nc, in_maps, list(range(8)), trace=trace)
    if trace:
        kernel.last_results = res

    out = np.empty((B, T, C), dtype=np.float32)
    for d in range(2):
        acc = res.results[4 * d]["outp"].astype(np.float32).copy()
        for g in range(1, G):
            acc += res.results[4 * d + g]["outp"].astype(np.float32)
        out[2 * d : 2 * d + 2] = acc + b_proj[None, None, :]
    return out


# revision 36
# speedup vs baseline: 1.0502x; 1.0502x over previous
"""Causal GQA self-attention block (B=4, T=2048, C=1024, H=16, G=4) on 8
Trainium2 NeuronCores.

Sharding: core c = d*4+g  (d in {0,1} batch-DP, g in {0..3} kv-group TP).
Each core handles batches [2d, 2d+1], heads {g, g+4, g+8, g+12}, kv group g,
and produces a partial projection output; the host sums the 4 TP partials
per batch pair and adds the bias.

Per-core kernel (fp16 operands, fp32 PSUM accumulation):
  - fused QKV projection from pre-transposed x (host supplies x^T in fp16),
    x^T staged as 8 per-chunk tiles so the first projection chain starts as
    soon as chunk 0 lands (kills the startup DMA serialization)
  - scores computed transposed (S^T[tk,tq] = K Q^T) in 128x512 tiles,
    head-pair packed into the PE array via tile_position (contraction=64)
  - causal: block skip + column trim + multiplicative 0/1 band mask on the
    diagonal blocks (applied post-exp on DVE)
  - unnormalized softmax: exp on ACT (scale folded), denominator obtained
    by appending a ones-column to V in the P@V matmul (M=65)
  - V^T -> V transposes via the DMA XBAR (off the PE/DVE)
  - normalize via DVE reciprocal (both heads in one [2,NT] call) + gpsimd
    partition-broadcast + DVE mult
  - output projection on-device, fp16 partials DMA'd out; host sums in fp32
  - both batches emitted as one flat pipeline so batch 1's attention starts
    while batch 0's trailing projection chunks still run
"""

import os
import sys

sys.path.insert(0, "/opt/trn_rl_repo")

import numpy as np
from contextlib import ExitStack

import concourse.bass as bass
import concourse.mybir as mybir
import concourse.tile as tile
from concourse import bacc
from concourse.bass_utils import run_bass_kernel_spmd

# problem shape (hardcoded per contract)
B, T, C = 4, 2048, 1024
H, G = 16, 4
D = C // H  # 64

# per-core
B_LOC = 2        # batches per core
NPAIR = 2        # head pairs per core (4 heads)
P = 128
CC = C // P      # 8 contraction chunks for projections
NT = 512         # tq tile width
TQT = T // NT    # 4 tq tiles
TKC = T // P     # 16 tk chunks

F32 = mybir.dt.float32
F16 = mybir.dt.float16
ADT = F16
Exp = mybir.ActivationFunctionType.Exp
ADD = mybir.AluOpType.add
MULT = mybir.AluOpType.mult


def _build_program():
    nc = bacc.Bacc(None, target_bir_lowering=False)

    xT = nc.dram_tensor("xT", [B_LOC, C, T], ADT, kind="ExternalInput")
    # pre-swizzled on host so each SBUF partition's data is contiguous in
    # DRAM (dense 6KB/4KB descriptors instead of a 768B trickle):
    # wqkv[p, cc, :] = WqkvT[cc*128+p, :] with columns q0|q1|k|v
    wqkv = nc.dram_tensor("wqkv", [P, CC, 384], ADT, kind="ExternalInput")
    wproj = nc.dram_tensor("wproj", [P, 2, C], ADT, kind="ExternalInput")
    # multiplicative triangular band mask, duplicated for the 2 packed heads
    maskb = nc.dram_tensor("maskb", [P, 2, P], ADT, kind="ExternalInput")
    ident2 = nc.dram_tensor("ident2", [P, 64], ADT, kind="ExternalInput")
    outp = nc.dram_tensor("outp", [B_LOC, T, C], F16, kind="ExternalOutput")

    with tile.TileContext(nc) as tc:
        with ExitStack() as ctx:
            const = ctx.enter_context(tc.tile_pool(name="const", bufs=1))
            xp = ctx.enter_context(tc.tile_pool(name="xp", bufs=2))
            sb2 = ctx.enter_context(tc.tile_pool(name="sb2", bufs=2))
            small = ctx.enter_context(tc.tile_pool(name="small", bufs=4))
            ppool = ctx.enter_context(tc.tile_pool(name="ppool", bufs=4))
            pvsp = ctx.enter_context(tc.tile_pool(name="pvsp", bufs=2))
            stg = ctx.enter_context(tc.tile_pool(name="stg", bufs=3))
            ps_st = ctx.enter_context(tc.tile_pool(name="ps_st", bufs=2, space="PSUM"))
            ps_pv = ctx.enter_context(tc.tile_pool(name="ps_pv", bufs=2, space="PSUM"))
            ps_mm = ctx.enter_context(tc.tile_pool(name="ps_mm", bufs=2, space="PSUM"))

            # ---- constants (single strided DMAs to cut issue latency) ----
            wqkv_t = const.tile([P, CC, 384], ADT, tag="wqkv")
            wproj_t = const.tile([P, 2, C], ADT, tag="wproj")
            mask_t = const.tile([P, 2, P], ADT, tag="maskb")
            id2_t = const.tile([P, 64], ADT, tag="ident2")

            def emit_consts_late():
                nc.scalar.dma_start(wproj_t[:], wproj[:])
                nc.scalar.dma_start(id2_t[:], ident2[:])

            def emit_setup(b, first=False):
                if first:
                    # the first projection chains need wqkv before anything
                    nc.scalar.dma_start(wqkv_t[:], wqkv[:])
                    nc.gpsimd.dma_start(mask_t[:], maskb[:])
                xts = []
                for cc in range(CC):
                    xt_c = xp.tile([P, T], ADT, tag=f"xt{cc}", name=f"xt{b}_{cc}")
                    eng = (nc.sync, nc.gpsimd, nc.scalar)[cc % 3]
                    eng.dma_start(xt_c[:], xT[b, cc * P : (cc + 1) * P, :])
                    xts.append(xt_c)
                if first:
                    emit_consts_late()
                # q_sb[:, p, t]: pair p -> heads (2p, 2p+1) at rows 0:64 / 64:128
                q_sb = sb2.tile([P, NPAIR, T], ADT, tag="q", name=f"q{b}")
                # kv_sb rows 0:64 = K^T (kv-group), rows 64:128 = V^T
                kv_sb = sb2.tile([P, TQT, NT], ADT, tag="kv", name=f"kv{b}")
                k_hi = sb2.tile([P, TQT, NT], ADT, tag="khi", name=f"khi{b}")
                v_a = sb2.tile([P, TKC, 65], ADT, tag="va", name=f"va{b}")
                nc.vector.memset(v_a[:, :, 64], 1.0)
                o_t = sb2.tile([P, NPAIR, T], ADT, tag="ot", name=f"ot{b}")
                return xts, q_sb, kv_sb, k_hi, v_a, o_t

            def emit_qkv_part(b, st8, n, part):
                # ---- QKV projection tile n, sub-part (0: kv proj + V
                # transpose + k dup, 1: q pair0 proj, 2: q pair1 proj) ----
                xts, q_sb, kv_sb, k_hi, v_a, o_t = st8
                m = {0: 2, 1: 0, 2: 1}[part]
                pm = ps_mm.tile([P, NT], F32, tag="mm")
                for cc in range(CC):
                    nc.tensor.matmul(
                        pm[:],
                        wqkv_t[:, cc, m * P : (m + 1) * P],
                        xts[cc][:, n * NT : (n + 1) * NT],
                        start=(cc == 0),
                        stop=(cc == CC - 1),
                    )
                if m < 2:
                    nc.vector.tensor_copy(q_sb[:, m, n * NT : (n + 1) * NT], pm[:])
                    # V transposes deferred here so they don't sit ahead of
                    # the q chains (and the first scores) in the PE queue
                    for i in range(4 * n + 2 * m, 4 * n + 2 * m + 2):
                        pt = ps_mm.tile([P, 64], ADT, tag="mm")
                        nc.tensor.transpose(
                            pt[:],
                            kv_sb[64:128, i // 4, (i % 4) * P : (i % 4 + 1) * P],
                            id2_t[64:128, :],
                        )
                        nc.vector.tensor_copy(v_a[:, i, 0:64], pt[:])
                    return
                nc.vector.tensor_copy(kv_sb[:, n, :], pm[:])
                nc.sync.dma_start(k_hi[64:128, n, :], kv_sb[0:64, n, :])

            def emit_attn_jp(b, st8, j, p_, fills=()):
                xts, q_sb, kv_sb, k_hi, v_a, o_t = st8
                fills = list(fills)
                pv = [
                    ps_pv.tile([P, NT], F32, tag="pv", name=f"pv{b}{j}{p_}{e}")
                    for e in range(2)
                ]
                last = 4 * j + 3
                for i in range(4 * j + 4):
                    diag = i >= 4 * j
                    r = i - 4 * j
                    lo = r * P if diag else 0
                    st = ps_st.tile([P, 2, NT], F32, tag="st")
                    for e in range(2):
                        ksrc = kv_sb if e == 0 else k_hi
                        nc.tensor.matmul(
                            st[:, e, lo:NT],
                            ksrc[
                                64 * e : 64 * e + 64,
                                i // 4,
                                (i % 4) * P : (i % 4 + 1) * P,
                            ],
                            q_sb[
                                64 * e : 64 * e + 64,
                                p_,
                                j * NT + lo : (j + 1) * NT,
                            ],
                            start=True,
                            stop=True,
                            tile_position=(64 * e, 0),
                        )
                    pexp = ppool.tile([P, 2, NT], ADT, tag="pexp")
                    nc.scalar.activation(
                        pexp[:, :, lo:NT],
                        st[:, :, lo:NT],
                        Exp,
                        scale=0.125,
                    )
                    if diag:
                        nc.vector.tensor_tensor(
                            pexp[:, :, lo : lo + P],
                            pexp[:, :, lo : lo + P],
                            mask_t[:],
                            MULT,
                        )
                    for e in range(2):
                        nc.tensor.matmul(
                            pv[e][0:65, lo:NT],
                            v_a[:, i, :],
                            pexp[:, e, lo:NT],
                            start=(i == 0),
                            stop=(i == last),
                        )
                # normalize: o = pv[0:64] / pv[64].  The PSUM->SBUF copies run
                # on ACT so the pv banks release without waiting on the DVE
                # queue; both heads' reciprocals batch into one [2, NT] call.
                pvs = pvsp.tile([65, 2, NT], F32, tag="pvs", name=f"pvs{b}{j}{p_}")
                for e in range(2):
                    nc.vector.tensor_copy(pvs[:, e, :], pv[e][0:65, :])
                # reciprocal_approx_fast and partition_broadcast require
                # absolute partition 0 on HW: shift denominator rows down
                l0 = small.tile([2, NT], F32, tag="l0")
                nc.sync.dma_start(l0[:], pvs[64:65, :, :])
                rec = small.tile([2, NT], F32, tag="rec")
                nc.vector.reciprocal_approx_fast(rec[:], l0[:])
                r1 = small.tile([1, NT], F32, tag="r1")
                nc.sync.dma_start(r1[:], rec[1:2, :])
                bca0 = small.tile([64, NT], F32, tag="bca0")
                nc.gpsimd.partition_broadcast(bca0[:], rec[0:1, :])
                bca1 = small.tile([64, NT], F32, tag="bca1")
                nc.gpsimd.partition_broadcast(bca1[:], r1[:])
                nc.vector.tensor_tensor(
                    o_t[0:64, p_, j * NT : (j + 1) * NT],
                    pvs[0:64, 0, :],
                    bca0[:],
                    MULT,
                )
                otmp = small.tile([64, NT], ADT, tag="otmp")
                nc.vector.tensor_tensor(otmp[:], pvs[0:64, 1, :], bca1[:], MULT)
                nc.sync.dma_start(o_t[64:128, p_, j * NT : (j + 1) * NT], otmp[:])
                for f in fills:
                    f()

            def emit_proj_t(b, st8, t_, tail=False):
                # ---- output projection for one tq chunk (fp16 partial) ----
                o_t = st8[5]
                stage = stg.tile([P, C], ADT, tag="stage")
                for n2 in range(2):
                    pm = ps_mm.tile([P, NT], F32, tag="mm")
                    for cc2 in range(2):
                        nc.tensor.matmul(
                            pm[:],
                            o_t[:, cc2, t_ * P : (t_ + 1) * P],
                            wproj_t[:, cc2, n2 * NT : (n2 + 1) * NT],
                            start=(cc2 == 0),
                            stop=(cc2 == 1),
                        )
                    if tail:
                        # ACT is idle after the last exp; fp16 staging makes
                        # its copy precision moot
                        nc.scalar.copy(stage[:, n2 * NT : (n2 + 1) * NT], pm[:])
                    else:
                        nc.vector.tensor_copy(
                            stage[:, n2 * NT : (n2 + 1) * NT], pm[:]
                        )
                nc.gpsimd.dma_start(outp[b, t_ * P : (t_ + 1) * P, :], stage[:])

            # ---- flat two-batch pipeline ----
            QK = lambda b, n, p: ("qkv", b, n, p)
            PR = lambda b, t: ("proj", b, t)
            SU = lambda b: ("setup", b)
            plan = [
                (0, 0, 0, [QK(0, 1, 0)]),
                (0, 0, 1, [QK(0, 1, 1), QK(0, 1, 2)]),
                (0, 1, 0, [QK(0, 2, 0), QK(0, 2, 1)]),
                (0, 1, 1, [QK(0, 2, 2), QK(0, 3, 0), PR(0, 0), PR(0, 1)]),
                (0, 2, 0, [QK(0, 3, 1), QK(0, 3, 2), PR(0, 2), PR(0, 3)]),
                (0, 2, 1, [SU(1), QK(1, 0, 0), PR(0, 4)]),
                (0, 3, 0, [QK(1, 0, 1), QK(1, 0, 2), PR(0, 5)]),
                (0, 3, 1, [QK(1, 1, 0), PR(0, 6), PR(0, 7)]),
                (1, 0, 0, [QK(1, 1, 1), QK(1, 1, 2), PR(0, 8)]),
                (1, 0, 1, [QK(1, 2, 0), QK(1, 2, 1), PR(0, 9)]),
                (1, 1, 0, [QK(1, 2, 2), QK(1, 3, 0), PR(0, 10), PR(0, 11)]),
                (1, 1, 1, [QK(1, 3, 1), QK(1, 3, 2), PR(0, 12), PR(0, 13)]),
                (1, 2, 0, [PR(0, 14), PR(0, 15), PR(1, 0), PR(1, 1)]),
                (1, 2, 1, [PR(1, 2), PR(1, 3), PR(1, 4), PR(1, 5)]),
                (1, 3, 0, [PR(1, 6), PR(1, 7), PR(1, 8), PR(1, 9)]),
                (1, 3, 1, [PR(1, 10), PR(1, 11)]),
            ]
            st = {0: emit_setup(0, first=True)}
            for p in range(3):
                emit_qkv_part(0, st[0], 0, p)
            for b, j, p_, fills in plan:
                emit_attn_jp(b, st[b], j, p_)
                for f in fills:
                    if f[0] == "qkv":
                        emit_qkv_part(f[1], st[f[1]], f[2], f[3])
                    elif f[0] == "proj":
                        emit_proj_t(f[1], st[f[1]], f[2])
                    elif f[0] == "setup":
                        st[f[1]] = emit_setup(f[1])
            for t_ in range(12, 16):
                emit_proj_t(1, st[1], t_, tail=True)

    nc.compile()
    return nc


_NC = None


def _get_program():
    global _NC
    if _NC is None:
        _NC = _build_program()
    return _NC


def _host_inputs(x, Wq, Wkv, Wproj):
    """Shard + lay out inputs for the 8 cores."""
    adt_np = np.float16
    tri = np.where(
        np.arange(P)[:, None] <= np.arange(P)[None, :], 1.0, 0.0
    ).astype(np.float32)
    maskb = np.stack([tri, tri], axis=1).astype(adt_np)  # [128, 2, 128]
    ident2 = np.concatenate([np.eye(64, dtype=np.float32)] * 2, axis=0).astype(
        adt_np
    )  # [128, 64]

    in_maps = []
    for d in range(2):
        xT = x[2 * d : 2 * d + 2].transpose(0, 2, 1).astype(adt_np)
        for g in range(G):
            heads = [g, g + 4, g + 8, g + 12]
            wq_cols = np.concatenate(
                [Wq[h * D : (h + 1) * D, :] for h in heads], axis=0
            ).T  # [1024, 256]
            wk = Wkv[g * D : (g + 1) * D, :].T  # [1024, 64]
            wv = Wkv[G * D + g * D : G * D + (g + 1) * D, :].T
            wqkv = np.concatenate([wq_cols, wk, wv], axis=1)  # [1024, 384]
            # swizzle: partition-major so each partition's rows are dense
            wqkv_s = np.ascontiguousarray(
                wqkv.reshape(CC, P, 384).transpose(1, 0, 2)
            ).astype(adt_np)  # [128, CC, 384]
            ch = np.concatenate(
                [np.arange(h * D, (h + 1) * D) for h in heads]
            )
            wproj_full = np.ascontiguousarray(Wproj[:, ch].T)  # [256, 1024]
            wproj_s = np.ascontiguousarray(
                wproj_full.reshape(2, P, C).transpose(1, 0, 2)
            ).astype(adt_np)  # [128, 2, 1024]
            in_maps.append(
                {
                    "xT": xT,
                    "wqkv": wqkv_s,
                    "wproj": wproj_s,
                    "maskb": maskb,
                    "ident2": ident2,
                }
            )
    return in_maps


def kernel(x, Wq, Wkv, Wproj, b_proj):
    x = np.asarray(x, dtype=np.float32)
    Wq = np.asarray(Wq, dtype=np.float32)
    Wkv = np.asarray(Wkv, dtype=np.float32)
    Wproj = np.asarray(Wproj, dtype=np.float32)
    b_proj = np.asarray(b_proj, dtype=np.float32)

    nc = _get_program()
    in_maps = _host_inputs(x, Wq, Wkv, Wproj)
    trace = bool(int(os.environ.get("BASS_KERNEL_TRACE", "0")))
    res = run_bass_kernel_spmd(nc, in_maps, list(range(8)), trace=trace)
    if trace:
        kernel.last_results = res

    out = np.empty((B, T, C), dtype=np.float32)
    for d in range(2):
        acc = res.results[4 * d]["outp"].astype(np.float32).copy()
        for g in range(1, G):
            acc += res.results[4 * d + g]["outp"].astype(np.float32)
        out[2 * d : 2 * d + 2] = acc + b_proj[None, None, :]
    return out


# revision 39
# speedup vs baseline: 1.0504x; 1.0001x over previous
"""Causal GQA self-attention block (B=4, T=2048, C=1024, H=16, G=4) on 8
Trainium2 NeuronCores.

Sharding: core c = d*4+g  (d in {0,1} batch-DP, g in {0..3} kv-group TP).
Each core handles batches [2d, 2d+1], heads {g, g+4, g+8, g+12}, kv group g,
and produces a partial projection output; the host sums the 4 TP partials
per batch pair and adds the bias.

Per-core kernel (fp16 operands, fp32 PSUM accumulation):
  - fused QKV projection from pre-transposed x (host supplies x^T in fp16),
    x^T staged as 8 per-chunk tiles so the first projection chain starts as
    soon as chunk 0 lands (kills the startup DMA serialization)
  - scores computed transposed (S^T[tk,tq] = K Q^T) in 128x512 tiles,
    head-pair packed into the PE array via tile_position (contraction=64)
  - causal: block skip + column trim + multiplicative 0/1 band mask on the
    diagonal blocks (applied post-exp on DVE)
  - unnormalized softmax: exp on ACT (scale folded), denominator obtained
    by appending a ones-column to V in the P@V matmul (M=65)
  - V^T -> V transposes on the PE, deferred behind the q chains (the DMA
    XBAR transpose gives wrong results on HW despite simulating correctly)
  - normalize via DVE reciprocal (both heads in one [2,NT] call) + gpsimd
    partition-broadcast + DVE mult
  - output projection on-device, fp16 partials DMA'd out; host sums in fp32
  - both batches emitted as one flat pipeline so batch 1's attention starts
    while batch 0's trailing projection chunks still run
"""

import os
import sys

sys.path.insert(0, "/opt/trn_rl_repo")

import numpy as np
from contextlib import ExitStack

import concourse.bass as bass
import concourse.mybir as mybir
import concourse.tile as tile
from concourse import bacc
from concourse.bass_utils import run_bass_kernel_spmd

# problem shape (hardcoded per contract)
B, T, C = 4, 2048, 1024
H, G = 16, 4
D = C // H  # 64

# per-core
B_LOC = 2        # batches per core
NPAIR = 2        # head pairs per core (4 heads)
P = 128
CC = C // P      # 8 contraction chunks for projections
NT = 512         # tq tile width
TQT = T // NT    # 4 tq tiles
TKC = T // P     # 16 tk chunks

F32 = mybir.dt.float32
F16 = mybir.dt.float16
ADT = F16
Exp = mybir.ActivationFunctionType.Exp
ADD = mybir.AluOpType.add
MULT = mybir.AluOpType.mult


def _build_program():
    nc = bacc.Bacc(None, target_bir_lowering=False)

    xT = nc.dram_tensor("xT", [B_LOC, C, T], ADT, kind="ExternalInput")
    # pre-swizzled on host so each SBUF partition's data is contiguous in
    # DRAM (dense 6KB/4KB descriptors instead of a 768B trickle):
    # wqkv[p, cc, :] = WqkvT[cc*128+p, :] with columns q0|q1|k|v
    wqkv = nc.dram_tensor("wqkv", [P, CC, 384], ADT, kind="ExternalInput")
    wproj = nc.dram_tensor("wproj", [P, 2, C], ADT, kind="ExternalInput")
    # multiplicative triangular band mask, duplicated for the 2 packed heads
    maskb = nc.dram_tensor("maskb", [P, 2, P], ADT, kind="ExternalInput")
    ident2 = nc.dram_tensor("ident2", [P, 64], ADT, kind="ExternalInput")
    outp = nc.dram_tensor("outp", [B_LOC, T, C], F16, kind="ExternalOutput")

    with tile.TileContext(nc) as tc:
        with ExitStack() as ctx:
            const = ctx.enter_context(tc.tile_pool(name="const", bufs=1))
            xp = ctx.enter_context(tc.tile_pool(name="xp", bufs=2))
            sb2 = ctx.enter_context(tc.tile_pool(name="sb2", bufs=2))
            small = ctx.enter_context(tc.tile_pool(name="small", bufs=4))
            ppool = ctx.enter_context(tc.tile_pool(name="ppool", bufs=4))
            pvsp = ctx.enter_context(tc.tile_pool(name="pvsp", bufs=2))
            stg = ctx.enter_context(tc.tile_pool(name="stg", bufs=3))
            ps_st = ctx.enter_context(tc.tile_pool(name="ps_st", bufs=2, space="PSUM"))
            ps_pv = ctx.enter_context(tc.tile_pool(name="ps_pv", bufs=2, space="PSUM"))
            ps_mm = ctx.enter_context(tc.tile_pool(name="ps_mm", bufs=2, space="PSUM"))

            # ---- constants (single strided DMAs to cut issue latency) ----
            wqkv_t = const.tile([P, CC, 384], ADT, tag="wqkv")
            wproj_t = const.tile([P, 2, C], ADT, tag="wproj")
            mask_t = const.tile([P, 2, P], ADT, tag="maskb")
            id2_t = const.tile([P, 64], ADT, tag="ident2")

            def emit_consts_late():
                nc.scalar.dma_start(wproj_t[:], wproj[:])
                nc.scalar.dma_start(id2_t[:], ident2[:])

            def emit_setup(b, first=False):
                if first:
                    # the first projection chains need wqkv before anything
                    nc.scalar.dma_start(wqkv_t[:], wqkv[:])
                    nc.gpsimd.dma_start(mask_t[:], maskb[:])
                xts = []
                for cc in range(CC):
                    xt_c = xp.tile([P, T], ADT, tag=f"xt{cc}", name=f"xt{b}_{cc}")
                    if first:
                        # head-critical: hardware DGE queues only
                        eng = (nc.sync, nc.scalar)[cc % 2]
                    else:
                        eng = (nc.sync, nc.gpsimd, nc.scalar)[cc % 3]
                    eng.dma_start(xt_c[:], xT[b, cc * P : (cc + 1) * P, :])
                    xts.append(xt_c)
                if first:
                    emit_consts_late()
                # q_sb[:, p, t]: pair p -> heads (2p, 2p+1) at rows 0:64 / 64:128
                q_sb = sb2.tile([P, NPAIR, T], ADT, tag="q", name=f"q{b}")
                # kv_sb rows 0:64 = K^T (kv-group), rows 64:128 = V^T
                kv_sb = sb2.tile([P, TQT, NT], ADT, tag="kv", name=f"kv{b}")
                k_hi = sb2.tile([P, TQT, NT], ADT, tag="khi", name=f"khi{b}")
                v_a = sb2.tile([P, TKC, 65], ADT, tag="va", name=f"va{b}")
                nc.vector.memset(v_a[:, :, 64], 1.0)
                o_t = sb2.tile([P, NPAIR, T], ADT, tag="ot", name=f"ot{b}")
                return xts, q_sb, kv_sb, k_hi, v_a, o_t

            def emit_qkv_part(b, st8, n, part):
                # ---- QKV projection tile n, sub-part (0: kv proj + V
                # transpose + k dup, 1: q pair0 proj, 2: q pair1 proj) ----
                xts, q_sb, kv_sb, k_hi, v_a, o_t = st8
                m = {0: 2, 1: 0, 2: 1}[part]
                pm = ps_mm.tile([P, NT], F32, tag="mm")
                for cc in range(CC):
                    nc.tensor.matmul(
                        pm[:],
                        wqkv_t[:, cc, m * P : (m + 1) * P],
                        xts[cc][:, n * NT : (n + 1) * NT],
                        start=(cc == 0),
                        stop=(cc == CC - 1),
                    )
                if m < 2:
                    nc.vector.tensor_copy(q_sb[:, m, n * NT : (n + 1) * NT], pm[:])
                    # V transposes deferred here so they don't sit ahead of
                    # the q chains (and the first scores) in the PE queue
                    for i in range(4 * n + 2 * m, 4 * n + 2 * m + 2):
                        pt = ps_mm.tile([P, 64], ADT, tag="mm")
                        nc.tensor.transpose(
                            pt[:],
                            kv_sb[64:128, i // 4, (i % 4) * P : (i % 4 + 1) * P],
                            id2_t[64:128, :],
                        )
                        nc.vector.tensor_copy(v_a[:, i, 0:64], pt[:])
                    return
                nc.vector.tensor_copy(kv_sb[:, n, :], pm[:])
                nc.sync.dma_start(k_hi[64:128, n, :], kv_sb[0:64, n, :])

            def emit_attn_jp(b, st8, j, p_, fills=()):
                xts, q_sb, kv_sb, k_hi, v_a, o_t = st8
                fills = list(fills)
                pv = [
                    ps_pv.tile([P, NT], F32, tag="pv", name=f"pv{b}{j}{p_}{e}")
                    for e in range(2)
                ]
                last = 4 * j + 3
                for i in range(4 * j + 4):
                    diag = i >= 4 * j
                    r = i - 4 * j
                    lo = r * P if diag else 0
                    st = ps_st.tile([P, 2, NT], F32, tag="st")
                    for e in range(2):
                        ksrc = kv_sb if e == 0 else k_hi
                        nc.tensor.matmul(
                            st[:, e, lo:NT],
                            ksrc[
                                64 * e : 64 * e + 64,
                                i // 4,
                                (i % 4) * P : (i % 4 + 1) * P,
                            ],
                            q_sb[
                                64 * e : 64 * e + 64,
                                p_,
                                j * NT + lo : (j + 1) * NT,
                            ],
                            start=True,
                            stop=True,
                            tile_position=(64 * e, 0),
                        )
                    pexp = ppool.tile([P, 2, NT], ADT, tag="pexp")
                    nc.scalar.activation(
                        pexp[:, :, lo:NT],
                        st[:, :, lo:NT],
                        Exp,
                        scale=0.125,
                    )
                    if diag:
                        nc.vector.tensor_tensor(
                            pexp[:, :, lo : lo + P],
                            pexp[:, :, lo : lo + P],
                            mask_t[:],
                            MULT,
                        )
                    for e in range(2):
                        nc.tensor.matmul(
                            pv[e][0:65, lo:NT],
                            v_a[:, i, :],
                            pexp[:, e, lo:NT],
                            start=(i == 0),
                            stop=(i == last),
                        )
                # normalize: o = pv[0:64] / pv[64].  The PSUM->SBUF copies run
                # on ACT so the pv banks release without waiting on the DVE
                # queue; both heads' reciprocals batch into one [2, NT] call.
                pvs = pvsp.tile([65, 2, NT], F32, tag="pvs", name=f"pvs{b}{j}{p_}")
                for e in range(2):
                    nc.vector.tensor_copy(pvs[:, e, :], pv[e][0:65, :])
                # reciprocal_approx_fast and partition_broadcast require
                # absolute partition 0 on HW: shift denominator rows down
                l0 = small.tile([2, NT], F32, tag="l0")
                nc.sync.dma_start(l0[:], pvs[64:65, :, :])
                rec = small.tile([2, NT], F32, tag="rec")
                nc.vector.reciprocal_approx_fast(rec[:], l0[:])
                r1 = small.tile([1, NT], F32, tag="r1")
                nc.sync.dma_start(r1[:], rec[1:2, :])
                bca0 = small.tile([64, NT], F32, tag="bca0")
                nc.gpsimd.partition_broadcast(bca0[:], rec[0:1, :])
                bca1 = small.tile([64, NT], F32, tag="bca1")
                nc.gpsimd.partition_broadcast(bca1[:], r1[:])
                nc.vector.tensor_tensor(
                    o_t[0:64, p_, j * NT : (j + 1) * NT],
                    pvs[0:64, 0, :],
                    bca0[:],
                    MULT,
                )
                otmp = small.tile([64, NT], ADT, tag="otmp")
                nc.vector.tensor_tensor(otmp[:], pvs[0:64, 1, :], bca1[:], MULT)
                nc.sync.dma_start(o_t[64:128, p_, j * NT : (j + 1) * NT], otmp[:])
                for f in fills:
                    f()

            def emit_proj_t(b, st8, t_, tail=False):
                # ---- output projection for one tq chunk (fp16 partial) ----
                o_t = st8[5]
                stage = stg.tile([P, C], ADT, tag="stage")
                for n2 in range(2):
                    pm = ps_mm.tile([P, NT], F32, tag="mm")
                    for cc2 in range(2):
                        nc.tensor.matmul(
                            pm[:],
                            o_t[:, cc2, t_ * P : (t_ + 1) * P],
                            wproj_t[:, cc2, n2 * NT : (n2 + 1) * NT],
                            start=(cc2 == 0),
                            stop=(cc2 == 1),
                        )
                    if tail:
                        # ACT is idle after the last exp; fp16 staging makes
                        # its copy precision moot
                        nc.scalar.copy(stage[:, n2 * NT : (n2 + 1) * NT], pm[:])
                    else:
                        nc.vector.tensor_copy(
                            stage[:, n2 * NT : (n2 + 1) * NT], pm[:]
                        )
                eng = nc.sync if (tail and t_ % 2) else nc.gpsimd
                eng.dma_start(outp[b, t_ * P : (t_ + 1) * P, :], stage[:])

            # ---- flat two-batch pipeline ----
            QK = lambda b, n, p: ("qkv", b, n, p)
            PR = lambda b, t: ("proj", b, t)
            SU = lambda b: ("setup", b)
            plan = [
                (0, 0, 0, [QK(0, 1, 0)]),
                (0, 0, 1, [QK(0, 1, 1), QK(0, 1, 2)]),
                (0, 1, 0, [QK(0, 2, 0), QK(0, 2, 1)]),
                (0, 1, 1, [QK(0, 2, 2), QK(0, 3, 0), PR(0, 0), PR(0, 1)]),
                (0, 2, 0, [QK(0, 3, 1), QK(0, 3, 2), PR(0, 2), PR(0, 3)]),
                (0, 2, 1, [SU(1), QK(1, 0, 0), PR(0, 4)]),
                (0, 3, 0, [QK(1, 0, 1), QK(1, 0, 2), PR(0, 5)]),
                (0, 3, 1, [QK(1, 1, 0), PR(0, 6), PR(0, 7)]),
                (1, 0, 0, [QK(1, 1, 1), QK(1, 1, 2), PR(0, 8)]),
                (1, 0, 1, [QK(1, 2, 0), QK(1, 2, 1), PR(0, 9)]),
                (1, 1, 0, [QK(1, 2, 2), QK(1, 3, 0), PR(0, 10), PR(0, 11)]),
                (1, 1, 1, [QK(1, 3, 1), QK(1, 3, 2), PR(0, 12), PR(0, 13)]),
                (1, 2, 0, [PR(0, 14), PR(0, 15), PR(1, 0), PR(1, 1)]),
                (1, 2, 1, [PR(1, 2), PR(1, 3), PR(1, 4), PR(1, 5)]),
                (1, 3, 0, [PR(1, 6), PR(1, 7), PR(1, 8), PR(1, 9)]),
                (1, 3, 1, [PR(1, 10), PR(1, 11)]),
            ]
            st = {0: emit_setup(0, first=True)}
            for p in range(3):
                emit_qkv_part(0, st[0], 0, p)
            for b, j, p_, fills in plan:
                emit_attn_jp(b, st[b], j, p_)
                for f in fills:
                    if f[0] == "qkv":
                        emit_qkv_part(f[1], st[f[1]], f[2], f[3])
                    elif f[0] == "proj":
                        emit_proj_t(f[1], st[f[1]], f[2])
                    elif f[0] == "setup":
                        st[f[1]] = emit_setup(f[1])
            for t_ in range(12, 16):
                emit_proj_t(1, st[1], t_, tail=True)

    nc.compile()
    return nc


_NC = None


def _get_program():
    global _NC
    if _NC is None:
        _NC = _build_program()
    return _NC


def _host_inputs(x, Wq, Wkv, Wproj):
    """Shard + lay out inputs for the 8 cores."""
    adt_np = np.float16
    tri = np.where(
        np.arange(P)[:, None] <= np.arange(P)[None, :], 1.0, 0.0
    ).astype(np.float32)
    maskb = np.stack([tri, tri], axis=1).astype(adt_np)  # [128, 2, 128]
    ident2 = np.concatenate([np.eye(64, dtype=np.float32)] * 2, axis=0).astype(
        adt_np
    )  # [128, 64]

    in_maps = []
    for d in range(2):
        xT = x[2 * d : 2 * d + 2].transpose(0, 2, 1).astype(adt_np)
        for g in range(G):
            heads = [g, g + 4, g + 8, g + 12]
            wq_cols = np.concatenate(
                [Wq[h * D : (h + 1) * D, :] for h in heads], axis=0
            ).T  # [1024, 256]
            wk = Wkv[g * D : (g + 1) * D, :].T  # [1024, 64]
            wv = Wkv[G * D + g * D : G * D + (g + 1) * D, :].T
            wqkv = np.concatenate([wq_cols, wk, wv], axis=1)  # [1024, 384]
            # swizzle: partition-major so each partition's rows are dense
            wqkv_s = np.ascontiguousarray(
                wqkv.reshape(CC, P, 384).transpose(1, 0, 2)
            ).astype(adt_np)  # [128, CC, 384]
            ch = np.concatenate(
                [np.arange(h * D, (h + 1) * D) for h in heads]
            )
            wproj_full = np.ascontiguousarray(Wproj[:, ch].T)  # [256, 1024]
            wproj_s = np.ascontiguousarray(
                wproj_full.reshape(2, P, C).transpose(1, 0, 2)
            ).astype(adt_np)  # [128, 2, 1024]
            in_maps.append(
                {
                    "xT": xT,
                    "wqkv": wqkv_s,
                    "wproj": wproj_s,
                    "maskb": maskb,
                    "ident2": ident2,
                }
            )
    return in_maps


def kernel(x, Wq, Wkv, Wproj, b_proj):
    x = np.asarray(x, dtype=np.float32)
    Wq = np.asarray(Wq, dtype=np.float32)
    Wkv = np.asarray(Wkv, dtype=np.float32)
    Wproj = np.asarray(Wproj, dtype=np.float32)
    b_proj = np.asarray(b_proj, dtype=np.float32)

    nc = _get_program()
    in_maps = _host_inputs(x, Wq, Wkv, Wproj)
    trace = bool(int(os.environ.get("BASS_KERNEL_TRACE", "0")))
    res = run_bass_kernel_spmd(nc, in_maps, list(range(8)), trace=trace)
    if trace:
        kernel.last_results = res

    out = np.empty((B, T, C), dtype=np.float32)
    for d in range(2):
        acc = res.results[4 * d]["outp"].astype(np.float32).copy()
        for g in range(1, G):
            acc += res.results[4 * d + g]["outp"].astype(np.float32)
        out[2 * d : 2 * d + 2] = acc + b_proj[None, None, :]
    return out
